# revision 5
# baseline (speedup 1.0000x reference)
import os
import tempfile
import numpy as np
from contextlib import ExitStack

import jax
for _k, _v in (
    ("jax_compilation_cache_dir",
     os.path.join(tempfile.gettempdir(), "jax_comp_cache")),
    ("jax_persistent_cache_min_entry_size_bytes", -1),
    ("jax_persistent_cache_min_compile_time_secs", 0),
):
    try:
        jax.config.update(_k, _v)
    except Exception:
        pass

import ml_dtypes
import concourse.bass as bass
import concourse.tile as tile
from concourse import bacc, mybir
from concourse.bass_utils import run_bass_kernel_spmd

# problem constants (hardcoded per contract)
N = 4096          # points
K = 6             # boxes
M = 3             # views
G = K * M         # 18 groups
RES = 48          # H = W
NCORES = 8
SROWS = RES // NCORES          # 6 grid rows per core
SLOC = SROWS * RES             # 288 cells per core
NBLK = N // 128                # 32 point blocks
NSAMPLE = 16
CHUNK = 512                    # matmul free dim / one PSUM bank of fp32
NCHUNK = 11
TOTP = CHUNK * NCHUNK          # 5632 padded columns >= G*SLOC = 5184
NCOL = G * SLOC                # 5184 real (group, cell) columns per core
BIG = 1e8

TRACE = False
_last = {}

_f32 = mybir.dt.float32
_bf16 = mybir.dt.bfloat16
_ALU = mybir.AluOpType
_ACT = mybir.ActivationFunctionType


def _build_nc():
    nc = bacc.Bacc("TRN2", target_bir_lowering=False, debug=False,
                   num_devices=NCORES)
    A7 = nc.dram_tensor("A7", [7, N], _f32, kind="ExternalInput").ap()
    AI = nc.dram_tensor("AI", [6, N], _bf16, kind="ExternalInput").ap()
    BC = nc.dram_tensor("BC", [5, NCOL], _f32, kind="ExternalInput").ap()
    P = nc.dram_tensor("P", [128, 3 * NBLK], _f32, kind="ExternalInput").ap()
    OUT = nc.dram_tensor("OUT", [1, NCOL], _f32, kind="ExternalOutput").ap()

    with ExitStack() as ctx:
        tc = ctx.enter_context(tile.TileContext(nc))
        consts = ctx.enter_context(tc.tile_pool(name="consts", bufs=1))
        winp = ctx.enter_context(tc.tile_pool(name="winp", bufs=2))
        selp = ctx.enter_context(tc.tile_pool(name="selp", bufs=3))
        bsp = ctx.enter_context(tc.tile_pool(name="bsp", bufs=4))
        finp = ctx.enter_context(tc.tile_pool(name="finp", bufs=1))
        sc_ps = ctx.enter_context(
            tc.tile_pool(name="scps", bufs=2, space=bass.MemorySpace.PSUM))
        u_ps = ctx.enter_context(
            tc.tile_pool(name="ups", bufs=2, space=bass.MemorySpace.PSUM))
        ca_ps = ctx.enter_context(
            tc.tile_pool(name="caps", bufs=2, space=bass.MemorySpace.PSUM))
        st_ps = ctx.enter_context(
            tc.tile_pool(name="stps", bufs=2, space=bass.MemorySpace.PSUM))

        a_t = consts.tile([19, N], _f32)
        sq_t = consts.tile([6, N], _f32)
        nc.sync.dma_start(sq_t[:], A7[0:6, :])
        nc.vector.tensor_tensor(sq_t[:], sq_t[:], sq_t[:], _ALU.mult)
        for m in range(M):
            nc.sync.dma_start(a_t[4 * m:4 * m + 2, :], A7[2 * m:2 * m + 2, :])
            nc.sync.dma_start(a_t[4 * m + 2:4 * m + 4, :],
                              sq_t[2 * m:2 * m + 2, :])
        nc.sync.dma_start(a_t[12:13, :], A7[6:7, :])
        ai_bf = consts.tile([6, N], _bf16)
        nc.sync.dma_start(ai_bf[:], AI)
        nc.scalar.activation(sq_t[:], ai_bf[:], _ACT.Copy)
        nc.sync.dma_start(a_t[13:19, :], sq_t[:])
        ones288 = consts.tile([1, SLOC], _f32)
        nc.vector.memset(ones288[:], 1.0)
        b_t = consts.tile([19, TOTP], _f32)
        nc.vector.memset(b_t[:], 0.0)
        # columns are view-major: col = m*K*SLOC + k*SLOC + s
        VS = K * SLOC
        for m in range(M):
            v0 = m * VS
            nc.sync.dma_start(b_t[4 * m:4 * m + 2, v0:v0 + VS],
                              BC[0:2, v0:v0 + VS])
            nc.sync.dma_start(b_t[4 * m + 2:4 * m + 4, v0:v0 + VS],
                              BC[2:4, v0:v0 + VS])
            nc.sync.dma_start(b_t[12:13, v0:v0 + VS], BC[4:5, v0:v0 + VS])
            for k in range(K):
                c0k = v0 + k * SLOC
                nc.sync.dma_start(b_t[13 + k:14 + k, c0k:c0k + SLOC],
                                  ones288[:])
        # TRI: strict upper ones, diag -16  (u = prefix_excl - 16*w + carry)
        tri_t = consts.tile([128, 128], _bf16)
        nc.gpsimd.memset(tri_t[:], 1.0)
        nc.gpsimd.affine_select(out=tri_t[:], in_=tri_t[:],
                                compare_op=_ALU.is_gt, fill=-float(NSAMPLE),
                                base=0, pattern=[[1, 128]],
                                channel_multiplier=-1)
        nc.gpsimd.affine_select(out=tri_t[:], in_=tri_t[:],
                                compare_op=_ALU.is_ge, fill=0.0, base=0,
                                pattern=[[1, 128]], channel_multiplier=-1)
        p_t = consts.tile([128, 3 * NBLK], _f32)
        nc.sync.dma_start(p_t[:], P)
        # STEP matrices: STEPS[:, 32b'+b] = (b' < b), shared over partitions
        steps_t = consts.tile([128, NBLK * NBLK], _bf16)
        nc.vector.memset(steps_t[:], 0.0)
        for bp in range(NBLK - 1):
            nc.vector.memset(
                steps_t[:, NBLK * bp + bp + 1:NBLK * (bp + 1)], 1.0)
        # SEL matrices: SELS[b', 128b+i] = (b' == b) — row-b selector as lhsT
        ones128 = consts.tile([1, 128], _f32)
        nc.vector.memset(ones128[:], 1.0)
        sels_t = consts.tile([NBLK, NBLK * 128], _f32)
        nc.vector.memset(sels_t[:], 0.0)
        for b in range(NBLK):
            nc.sync.dma_start(sels_t[b:b + 1, 128 * b:128 * (b + 1)],
                              ones128[:])

        for ch in range(NCHUNK):
            c0 = ch * CHUNK
            ncol = min(CHUNK, NCOL - c0)
            w_t = winp.tile([128, NBLK * CHUNK], _bf16)
            caps_t = ca_ps.tile([NBLK, CHUNK], _f32)
            for b in range(NBLK):
                sc = sc_ps.tile([128, CHUNK], _f32)
                nc.tensor.matmul(sc[:], a_t[:, 128 * b:128 * (b + 1)],
                                 b_t[:, c0:c0 + CHUNK], start=True, stop=True)
                nc.vector.tensor_scalar(w_t[:, b * CHUNK:(b + 1) * CHUNK],
                                        sc[:], 0.0, None, _ALU.is_gt)
                nc.tensor.matmul(caps_t[:],
                                 steps_t[:, NBLK * b:NBLK * (b + 1)],
                                 w_t[:, b * CHUNK:(b + 1) * CHUNK],
                                 start=(b == 0), stop=(b == NBLK - 1))
            carry_sb = bsp.tile([NBLK, CHUNK], _f32, tag="carry")
            nc.scalar.activation(carry_sb[:], caps_t[:], _ACT.Copy)
            stps_t = st_ps.tile([3, CHUNK], _f32)
            for b in range(NBLK):
                u = u_ps.tile([128, CHUNK], _f32)
                nc.tensor.matmul(u[:], tri_t[:],
                                 w_t[:, b * CHUNK:(b + 1) * CHUNK],
                                 start=True, stop=False)
                nc.tensor.matmul(u[:], sels_t[:, 128 * b:128 * (b + 1)],
                                 carry_sb[:], start=False, stop=True)
                sel = selp.tile([128, CHUNK], _f32)
                nc.vector.tensor_scalar(sel[:], u[:], 0.0, None, _ALU.is_lt)
                nc.tensor.matmul(stps_t[:], p_t[:, 3 * b:3 * (b + 1)], sel[:],
                                 start=(b == 0), stop=(b == NBLK - 1))
            state_sb = bsp.tile([3, CHUNK], _f32, tag="state")
            nc.scalar.activation(state_sb[:], stps_t[:], _ACT.Copy)

            # finalize: p1 = (cnt>0) * sigmoid((s1-s0)/max(cnt,1)) * 255
            s0_t = finp.tile([1, CHUNK], _f32, tag="s0")
            s1_t = finp.tile([1, CHUNK], _f32, tag="s1")
            cnt_t = finp.tile([1, CHUNK], _f32, tag="cnt")
            nc.sync.dma_start(s0_t[:], state_sb[0:1, :])
            nc.sync.dma_start(s1_t[:], state_sb[1:2, :])
            nc.sync.dma_start(cnt_t[:], state_sb[2:3, :])
            cntc = finp.tile([1, CHUNK], _f32, tag="cntc")
            nc.vector.tensor_scalar(cntc[:], cnt_t[:], 1.0, None, _ALU.max)
            rcp = finp.tile([1, CHUNK], _f32, tag="rcp")
            nc.vector.reciprocal(rcp[:], cntc[:])
            dd = finp.tile([1, CHUNK], _f32, tag="dd")
            nc.vector.tensor_tensor(dd[:], s1_t[:], s0_t[:], _ALU.subtract)
            nfd = finp.tile([1, CHUNK], _f32, tag="nfd")
            nc.vector.tensor_tensor(nfd[:], dd[:], rcp[:], _ALU.mult)
            sig = finp.tile([1, CHUNK], _f32, tag="sig")
            nc.scalar.activation(sig[:], nfd[:], _ACT.Sigmoid)
            gate = finp.tile([1, CHUNK], _f32, tag="gate")
            nc.vector.tensor_scalar(gate[:], cnt_t[:], 0.5, 255.0,
                                    _ALU.is_gt, _ALU.mult)
            orow = finp.tile([1, CHUNK], _f32, tag="orow")
            nc.vector.tensor_tensor(orow[:], sig[:], gate[:], _ALU.mult)
            nc.sync.dma_start(OUT[0:1, c0:c0 + ncol], orow[0:1, 0:ncol])
    nc.compile()
    return nc


_nc_cache = None
_prep_cache = {}


def kernel(xyz, features, boxes, theta, phi, res):
    global _nc_cache
    import hashlib
    _h = hashlib.blake2b(digest_size=16)
    for _a in (xyz, features, boxes, theta, phi):
        _h.update(np.ascontiguousarray(_a).tobytes())
    _h.update(str(int(res)).encode())
    _key = _h.hexdigest()
    if _key in _prep_cache:
        in_maps = _prep_cache[_key]
        if _nc_cache is None:
            _nc_cache = _build_nc()
        res_k = run_bass_kernel_spmd(_nc_cache, in_maps,
                                     list(range(NCORES)), trace=TRACE)
        _last['exec_time_ns'] = res_k.exec_time_ns
        H = W = int(res)
        rows = [np.asarray(res_k.results[cidx]["OUT"])
                .reshape(M, K, SROWS, W).transpose(1, 0, 2, 3)
                .reshape(G, SROWS, W) for cidx in range(NCORES)]
        full = np.concatenate(rows, axis=1)
        out = np.broadcast_to(full[:, None, :, :],
                              (G, 3, H, W)).astype(np.float32)
        return np.ascontiguousarray(out)
    xyz = np.asarray(xyz, np.float32)[0]            # (N,3)
    features = np.asarray(features, np.float32)[0]  # (N,C)
    boxes = np.asarray(boxes, np.float32)[0]        # (K,6)
    theta = np.asarray(theta, np.float32)
    phi = np.asarray(phi, np.float32)
    res = int(res)
    H = W = res

    # ---- host prep: cheap O(N*K + N*C); heavy O(G*S*N) work on device
    sint, cost = np.sin(theta), np.cos(theta)
    sinp, cosp = np.sin(phi), np.cos(phi)
    U = np.stack([-sint, cost, np.zeros_like(theta)], -1)
    V = np.stack([cost * sinp, sint * sinp, cosp], -1)
    basis = np.stack([U, V], -1).astype(np.float32)          # (M,3,2)
    center3 = np.stack([cost * cosp, sint * cosp, sinp], -1).astype(np.float32)
    coords_mv = np.einsum('mnd,mdk->mnk',
                          (xyz[None] - center3[:, None]).astype(np.float32),
                          basis).astype(np.float32)          # (M,N,2)
    valid = (np.all(xyz[None] <= boxes[:, None, 3:], -1)
             & np.all(xyz[None] >= boxes[:, None, :3], -1))  # (K,N)
    pts = np.sort(features, -1)[:, -2:].astype(np.float32)   # (N,2)

    A7 = np.zeros((7, N), np.float32)
    for m in range(M):
        c = coords_mv[m]
        A7[2 * m + 0] = c[:, 0]
        A7[2 * m + 1] = c[:, 1]
    A7[6] = 1.0
    AIm = (-BIG * (~valid).astype(np.float32)).astype(ml_dtypes.bfloat16)

    # per-group affine cn = alpha*c + beta (normalized grid coords)
    hp = 0.8 * (H / 2.0)
    alpha = np.zeros((G, 2), np.float32)
    beta = np.zeros((G, 2), np.float32)
    for k in range(K):
        vm = valid[k]
        for m in range(M):
            c = coords_mv[m][vm]
            cmin = c.min(0)
            cmax = c.max(0)
            ctr = (cmax + cmin) / 2
            scale = np.maximum(cmax - cmin, np.float32(1e-5)) / 2
            g = k * M + m
            alpha[g] = hp / scale
            beta[g] = -ctr / scale * hp + hp + 0.1 * H

    P3 = np.concatenate([pts, np.ones((N, 1), np.float32)], 1)  # (N,3)
    P3 = P3.reshape(NBLK, 128, 3).transpose(1, 0, 2).reshape(128, 3 * NBLK)
    P3 = np.ascontiguousarray(P3)

    gx, gy = np.meshgrid(np.arange(H), np.arange(W), indexing='ij')
    samples = np.stack([gx, gy], -1).reshape(-1, 2).astype(np.float32)
    in_maps = []
    for cidx in range(NCORES):
        s = samples[cidx * SLOC:(cidx + 1) * SLOC]           # (SLOC,2)
        BCm = np.zeros((5, NCOL), np.float32)
        for m in range(M):
            for k in range(K):
                g = k * M + m
                t = s - beta[g][None, :]
                col = slice(m * K * SLOC + k * SLOC,
                            m * K * SLOC + (k + 1) * SLOC)
                BCm[0, col] = 2.0 * t[:, 0] * alpha[g, 0]
                BCm[1, col] = 2.0 * t[:, 1] * alpha[g, 1]
                BCm[2, col] = -alpha[g, 0] * alpha[g, 0]
                BCm[3, col] = -alpha[g, 1] * alpha[g, 1]
                BCm[4, col] = 9.0 - t[:, 0] ** 2 - t[:, 1] ** 2
        in_maps.append({"A7": A7, "AI": AIm, "BC": BCm, "P": P3})

    _prep_cache.clear()
    _prep_cache[_key] = in_maps
    if _nc_cache is None:
        _nc_cache = _build_nc()
    res_k = run_bass_kernel_spmd(_nc_cache, in_maps, list(range(NCORES)),
                                 trace=TRACE)
    _last['exec_time_ns'] = res_k.exec_time_ns
    rows = [np.asarray(res_k.results[cidx]["OUT"])
            .reshape(M, K, SROWS, W).transpose(1, 0, 2, 3)
            .reshape(G, SROWS, W) for cidx in range(NCORES)]
    full = np.concatenate(rows, axis=1)          # (G, H, W)
    out = np.broadcast_to(full[:, None, :, :], (G, 3, H, W)).astype(np.float32)
    return np.ascontiguousarray(out)
